# revision 18
# baseline (speedup 1.0000x reference)
"""Trainium2 Bass kernel for CaptionAttentionNet.

Model (B=128, T=64, V=10000, E=512, D=512, F=2048):
  h/c inits from image vectors; x = emb[captions_ix]
  h1s = LSTM1(x);  attn1 = out_proj1(v_proj1(h1s))        (softmax over 1 key == 1)
  h2s = LSTM2([h1s, attn1]);  attn2 = out_proj2(v_proj2(h2s))
  logits = [h2s, attn1, attn2] @ W_logits.T + b_logits

Since the "attention" is affine (single key), it folds into the weights on the
host:  attn_i = h_is @ M_i.T + a_i  with  M_i = Wo_i @ Wv_i.  LSTM2's input
projection becomes h1s @ Weff.T and the logits collapse to
h1s @ G1.T + h2s @ G2.T + b_eff.  The device computes, per core (16 batch rows):
  xp1 = x @ W_ih1r.T + b1          (bulk matmul)
  LSTM1 recurrence (64 steps)      -> h1sT in SBUF
  xp2 = h1s @ Weff.T + b2eff       (bulk matmul)
  LSTM2 recurrence (64 steps)      -> h2sT in SBUF
  logits = h1s @ G1.T + h2s @ G2.T + b_eff   (vocab-chunked)

Everything on device is kept feature-major ("transposed"): the recurrence
computes gatesT [2048, 16] with W_hhT tiles stationary, so the elementwise
gate math runs on all 128 partitions and h lands directly in the layout the
next step / the logits matmuls consume.  Column index everywhere: t*16 + b.
Gate blocks are reordered (i, f, o, g) so one sigmoid covers i|f|o.
"""

import os

# The device program runs through jax/PJRT on the axon/neuron platform; a
# JAX_PLATFORMS=cpu inherited from a reference-side harness would hide it.
if os.environ.get("JAX_PLATFORMS") == "cpu":
    os.environ.pop("JAX_PLATFORMS")

import numpy as np
import ml_dtypes

BF16 = ml_dtypes.bfloat16

B, T, V, E, D, F = 128, 64, 10000, 512, 512, 2048
NCORES = 8
BC = B // NCORES  # 16 batch rows per core
R = BC * T  # 1024 (t-major rows per core)
VP = 10240  # padded vocab
NV = VP // 512  # 20 vocab chunks
G4 = 4 * D  # 2048 gates

_GATE_PERM = [0, 1, 3, 2]  # (i, f, g, o) -> (i, f, o, g)


def _reorder_gates(w):
    """Reorder the leading 4*D gate axis from (i,f,g,o) to (i,f,o,g)."""
    return w.reshape(4, D, *w.shape[1:])[_GATE_PERM].reshape(4 * D, *w.shape[1:])


def _tt(w):
    """[G, K] -> [128, K//128, G] transposed k-chunk tiles (lhsT layout)."""
    g, k = w.shape
    return np.ascontiguousarray(w.T.reshape(k // 128, 128, g).transpose(1, 0, 2))


def _bt(v):
    """[BC, 512] -> [128, 4, BC] transposed chunk tiles."""
    return np.ascontiguousarray(v.T.reshape(4, 128, v.shape[0]).transpose(1, 0, 2))


def _host_prep(inputs):
    f32 = np.float32
    inp = {k: np.asarray(v) for k, v in inputs.items()}

    emb = inp["emb"].astype(f32)
    ix = inp["captions_ix"].astype(np.int64)
    img = inp["image_vectors"].astype(f32)

    x = emb[ix]  # [B, T, E]

    Wo1, Wv1 = inp["Wo1"].astype(f32), inp["Wv1"].astype(f32)
    Wo2, Wv2 = inp["Wo2"].astype(f32), inp["Wv2"].astype(f32)
    M1 = Wo1 @ Wv1
    a1b = inp["bo1"].astype(f32) + Wo1 @ inp["bv1"].astype(f32)
    M2 = Wo2 @ Wv2
    a2b = inp["bo2"].astype(f32) + Wo2 @ inp["bv2"].astype(f32)

    W_ih2 = inp["W_ih2"].astype(f32)
    Wa, Wb = W_ih2[:, :D], W_ih2[:, D:]
    Weff2 = Wa + Wb @ M1
    b2e = inp["b2"].astype(f32) + Wb @ a1b

    W_logits = inp["W_logits"].astype(f32)
    Wla, Wlb, Wlc = W_logits[:, :D], W_logits[:, D : 2 * D], W_logits[:, 2 * D :]
    G1 = Wlb @ M1
    G2 = Wla + Wlc @ M2
    blog = inp["b_logits"].astype(f32) + Wlb @ a1b + Wlc @ a2b

    h10 = img @ inp["W_init_h1"].astype(f32).T + inp["b_init_h1"].astype(f32)
    c10 = img @ inp["W_init_c1"].astype(f32).T + inp["b_init_c1"].astype(f32)
    h20 = img @ inp["W_init_h2"].astype(f32).T + inp["b_init_h2"].astype(f32)
    c20 = img @ inp["W_init_c2"].astype(f32).T + inp["b_init_c2"].astype(f32)

    wih1r = _reorder_gates(inp["W_ih1"].astype(f32))
    whh1r = _reorder_gates(inp["W_hh1"].astype(f32))
    whh2r = _reorder_gates(inp["W_hh2"].astype(f32))
    weff2r = _reorder_gates(Weff2)
    b1r = _reorder_gates(inp["b1"].astype(f32)[:, None])[:, 0]
    b2r = _reorder_gates(b2e[:, None])[:, 0]

    # Padded G tiles: [NV, 128, 8, 512]; kc<4 -> G1 d-chunk, kc>=4 -> G2 d-chunk
    G1p = np.zeros((VP, D), f32)
    G1p[:V] = G1
    G2p = np.zeros((VP, D), f32)
    G2p[:V] = G2
    blogp = np.zeros((VP,), f32)
    blogp[:V] = blog

    def gtiles(G):
        # [VP, D] -> [NV, 128, 4, 512] with [v, p, dc, n] = G[v*512+n, dc*128+p]
        return G.T.reshape(4, 128, NV, 512).transpose(2, 1, 0, 3)

    g12t = np.ascontiguousarray(
        np.concatenate([gtiles(G1p), gtiles(G2p)], axis=2)
    ).astype(BF16)

    shared = {
        "wih1t": _tt(wih1r).astype(BF16),
        "whh1t": _tt(whh1r).astype(BF16),
        "weff2t": _tt(weff2r).astype(BF16),
        "whh2t": _tt(whh2r).astype(BF16),
        "b1g": np.ascontiguousarray(b1r.reshape(16, 128).T).astype(f32),
        "b2g": np.ascontiguousarray(b2r.reshape(16, 128).T).astype(f32),
        "g12t": g12t,
    }

    per_core = []
    for c in range(NCORES):
        sl = slice(c * BC, (c + 1) * BC)
        xs = x[sl]  # [BC, T, E]
        # t-major rows: row = t*BC + b
        xr = np.ascontiguousarray(xs.transpose(1, 0, 2)).reshape(R, E)
        xt = np.ascontiguousarray(xr.T.reshape(4, 128, R).transpose(1, 0, 2))
        per_core.append(
            {
                "xt": xt.astype(BF16),
                "h1p0": _bt(h10[sl]).astype(BF16),
                "h2p0": _bt(h20[sl]).astype(BF16),
                "c10": _bt(c10[sl]).astype(f32),
                "c20": _bt(c20[sl]).astype(f32),
                **shared,
            }
        )
    return per_core, blog


def build_program(nc):
    """Emit the full per-core program into `nc` (Bacc). Same program all cores."""
    import concourse.tile as tile
    from concourse import mybir

    dt = mybir.dt
    AF = mybir.ActivationFunctionType

    def din(name, shape, dtype=dt.bfloat16):
        return nc.dram_tensor(name, shape, dtype, kind="ExternalInput").ap()

    xt_d = din("xt", [128, 4, R])
    wih1t_d = din("wih1t", [128, 4, G4])
    whh1t_d = din("whh1t", [128, 4, G4])
    weff2t_d = din("weff2t", [128, 4, G4])
    whh2t_d = din("whh2t", [128, 4, G4])
    b1g_d = din("b1g", [128, 16], dt.float32)
    b2g_d = din("b2g", [128, 16], dt.float32)
    h1p0_d = din("h1p0", [128, 4, BC])
    h2p0_d = din("h2p0", [128, 4, BC])
    c10_d = din("c10", [128, 4, BC], dt.float32)
    c20_d = din("c20", [128, 4, BC], dt.float32)
    g12t_d = din("g12t", [NV, 128, 8, 512])
    out_d = nc.dram_tensor("out", [R, V], dt.float32, kind="ExternalOutput").ap()

    with tile.TileContext(nc) as tc:
        with (
            tc.tile_pool(name="const", bufs=1) as const,
            tc.tile_pool(name="state", bufs=1) as state,
            tc.tile_pool(name="work", bufs=5) as work,
            tc.tile_pool(name="gbuf", bufs=3) as gbuf,
            tc.tile_pool(name="obuf", bufs=4) as obuf,
            tc.tile_pool(name="pg", bufs=4, space="PSUM") as pg,
            tc.tile_pool(name="pl", bufs=4, space="PSUM") as pl,
        ):
            # ---- persistent SBUF tensors ----
            def load(pool, d_ap, shape, dtype=dt.bfloat16, tag=None):
                t = pool.tile(shape, dtype, tag=tag)
                nc.sync.dma_start(out=t[:], in_=d_ap)
                return t

            # order matters: everything LSTM1 step 0 needs comes first
            b1g = load(const, b1g_d[:], [128, 16], dt.float32, tag="b1g")
            h1p0 = load(const, h1p0_d[:], [128, 4, BC], tag="h1p0")
            xt = load(const, xt_d[:], [128, 4, R], tag="xt")
            wih1t = load(const, wih1t_d[:], [128, 4, G4], tag="wih1t")
            whh1t = load(const, whh1t_d[:], [128, 4, G4], tag="whh1t")
            weff2t = load(const, weff2t_d[:], [128, 4, G4], tag="weff2t")
            whh2t = load(const, whh2t_d[:], [128, 4, G4], tag="whh2t")
            b2g = load(const, b2g_d[:], [128, 16], dt.float32, tag="b2g")
            h2p0 = load(const, h2p0_d[:], [128, 4, BC], tag="h2p0")

            xp1t = state.tile([128, 16, R], dt.bfloat16, tag="xp1t")
            xp2t = state.tile([128, 16, R], dt.bfloat16, tag="xp2t")
            h1st = state.tile([128, 4, R], dt.bfloat16, tag="h1st")
            h2st = state.tile([128, 4, R], dt.bfloat16, tag="h2st")
            c1 = load(state, c10_d[:], [128, 4, BC], dt.float32, tag="c1")
            c2 = load(state, c20_d[:], [128, 4, BC], dt.float32, tag="c2")

            # ---- input projection for a column range: xpT[g, cols] = W @ rhs + b
            def xp_cols(wt, rhs_tile, bg, xpt, c0, c1_):
                # one [128, 512] psum bank covers 512/cw gate-tiles' worth of cols
                cw = c1_ - c0
                per = 512 // cw
                for gq in range(16 // per):
                    ps = pl.tile([128, 512], dt.float32, tag="pl")
                    for gi in range(per):
                        gb = gq * per + gi
                        gsl = slice(gb * 128, (gb + 1) * 128)
                        psl = slice(gi * cw, (gi + 1) * cw)
                        for dc in range(4):
                            nc.tensor.matmul(
                                ps[:, psl],
                                wt[:, dc, gsl],
                                rhs_tile[:, dc, c0:c1_],
                                start=(dc == 0),
                                stop=(dc == 3),
                            )
                    for gi in range(per):
                        gb = gq * per + gi
                        nc.scalar.activation(
                            xpt[:, gb, c0:c1_],
                            ps[:, gi * cw : (gi + 1) * cw],
                            AF.Identity,
                            bias=bg[:, gb : gb + 1],
                        )

            def xp_cols_q(wt, rhs_tile, bg, xpt, c0, c1_, gq):
                # one gate-quarter (4 gb tiles) over cols [c0, c1_), width 128
                cw = c1_ - c0
                ps = pl.tile([128, 512], dt.float32, tag="pl")
                for gi in range(4):
                    gb = gq * 4 + gi
                    gsl = slice(gb * 128, (gb + 1) * 128)
                    psl = slice(gi * cw, (gi + 1) * cw)
                    for dc in range(4):
                        nc.tensor.matmul(
                            ps[:, psl],
                            wt[:, dc, gsl],
                            rhs_tile[:, dc, c0:c1_],
                            start=(dc == 0),
                            stop=(dc == 3),
                        )
                for gi in range(4):
                    gb = gq * 4 + gi
                    nc.scalar.activation(
                        xpt[:, gb, c0:c1_],
                        ps[:, gi * cw : (gi + 1) * cw],
                        AF.Identity,
                        bias=bg[:, gb : gb + 1],
                    )

            # ---- one LSTM recurrence step ----
            def lstm_step(t_, whht, xpt, hst, h_prev_ap, c):
                ps = pg.tile([128, 16, BC], dt.float32, tag="pg")
                for gb in range(16):
                    gsl = slice(gb * 128, (gb + 1) * 128)
                    for dc in range(4):
                        nc.tensor.matmul(
                            ps[:, gb, :],
                            whht[:, dc, gsl],
                            h_prev_ap[:, dc, :],
                            start=(dc == 0),
                            stop=(dc == 3),
                        )
                # gates layout: [128, 16, BC] = (gate-tile, batch); i|f|o|g in 4-tile groups
                gs = work.tile([128, 16, BC], dt.float32, tag="gs")
                nc.vector.tensor_add(gs[:], ps[:], xpt[:, :, t_ * BC : (t_ + 1) * BC])
                ss = work.tile([128, 12, BC], dt.float32, tag="ss")
                nc.scalar.activation(ss[:], gs[:, :12, :], AF.Sigmoid)
                tg = work.tile([128, 4, BC], dt.float32, tag="tg")
                nc.scalar.activation(tg[:], gs[:, 12:, :], AF.Tanh)
                t1 = work.tile([128, 4, BC], dt.float32, tag="t1")
                nc.vector.tensor_mul(t1[:], ss[:, 4:8, :], c[:])
                t2 = work.tile([128, 4, BC], dt.float32, tag="t2")
                nc.vector.tensor_mul(t2[:], ss[:, :4, :], tg[:])
                nc.vector.tensor_add(c[:], t1[:], t2[:])
                tc_ = work.tile([128, 4, BC], dt.float32, tag="tc")
                nc.scalar.activation(tc_[:], c[:], AF.Tanh)
                nc.vector.tensor_mul(
                    hst[:, :, t_ * BC : (t_ + 1) * BC], ss[:, 8:12, :], tc_[:]
                )

            # ---- one logits unit: psum[128 rows, 512 vocab] for (v, m) ----
            def logits_unit(v, m, gt):
                width = min(512, V - v * 512)
                ps = pl.tile([128, 512], dt.float32, tag="pl")
                msl = slice(m * 128, (m + 1) * 128)
                for kc in range(8):
                    hs = h1st if kc < 4 else h2st
                    nc.tensor.matmul(
                        ps[:],
                        hs[:, kc % 4, msl],
                        gt[:, kc, :],
                        start=(kc == 0),
                        stop=(kc == 7),
                    )
                ot = obuf.tile([128, 512], dt.float32, tag="ot")
                if m % 2 == 0:
                    nc.scalar.copy(ot[:], ps[:])
                else:
                    nc.vector.tensor_copy(ot[:], ps[:])
                nc.sync.dma_start(
                    out=out_d[msl, v * 512 : v * 512 + width],
                    in_=ot[:, :width],
                )

            # phase 1: xp1 for the first two step-blocks only; the rest is
            # deferred into the recurrence stalls via the fill queue.
            xp_cols(wih1t, xt, b1g, xp1t, 0, 256)

            # phase 2: LSTM1 / xp2 / LSTM2 interleaved, L2 lagging one
            # 8-step block so each LSTM's elementwise chain hides under the
            # other's matmuls and the PE stays dense.  Early logits units
            # (row-blocks already finished by L2) are drip-fed one per step
            # pair to fill the PE stalls left by the elementwise chains.
            SB = 8  # steps per block
            NBLK = T // SB

            def l1_step(t_):
                hp = h1p0[:, :, :] if t_ == 0 else h1st[:, :, (t_ - 1) * BC : t_ * BC]
                lstm_step(t_, whh1t, xp1t, h1st, hp, c1)

            def l2_step(t_):
                hp = h2p0[:, :, :] if t_ == 0 else h2st[:, :, (t_ - 1) * BC : t_ * BC]
                lstm_step(t_, whh2t, xp2t, h2st, hp, c2)

            # ---- static fill schedule: one unit per step-pair period ----
            # slot s runs L1 block s and L2 block s-1; logits row-block m is
            # ready from slot m+2 on.
            fill_by_slot = [[] for _ in range(NBLK + 1)]
            # deferred xp1: blocks 2..7, four gate-quarter units each
            for blk in range(2, NBLK):
                slot = 0 if blk < 6 else 1
                for gq in range(4):
                    fill_by_slot[slot].append(("xp1", blk, gq))
            # early logits: pack v-contiguous runs of ready row-blocks
            done_units = set()
            nm = [0] * NV
            vptr = 0
            cap = [0, 0, 8, 8, 8, 8, 8, 8, 16]
            for s in range(2, NBLK + 1):
                placed = len(fill_by_slot[s])
                while placed < cap[s] and vptr < NV:
                    lim = min(s - 1, R // 128)
                    if nm[vptr] >= lim:
                        vptr += 1  # rest of this v goes to the tail phase
                        continue
                    m = nm[vptr]
                    fill_by_slot[s].append(("lg", vptr, m))
                    done_units.add((vptr, m))
                    nm[vptr] += 1
                    placed += 1
                # never resume a v in a later slot (keeps <=2 gt tiles live)
                if vptr < NV and nm[vptr] > 0:
                    vptr += 1

            gts = {}

            def emit_fill(u):
                if u[0] == "xp1":
                    _, blk, gq = u
                    c0 = blk * SB * BC
                    xp_cols_q(wih1t, xt, b1g, xp1t, c0, c0 + 128, gq)
                else:
                    _, v, m = u
                    if v not in gts:
                        gts[v] = gbuf.tile([128, 8, 512], dt.bfloat16, tag="gt", name=f"gt_e{v}")
                        nc.sync.dma_start(out=gts[v][:], in_=g12t_d[v])
                    logits_unit(v, m, gts[v])

            fill_queue = []
            for s in range(NBLK + 1):
                fill_queue.extend(fill_by_slot[s])
                per_period = 2 if (s < 2 or s >= NBLK) else 1
                for i in range(SB):
                    if s < NBLK:
                        l1_step(s * SB + i)
                    if s > 0:
                        l2_step((s - 1) * SB + i)
                    for _ in range(per_period):
                        if fill_queue:
                            emit_fill(fill_queue.pop(0))
                if s < NBLK:
                    # xp2 for the L1 block just produced
                    xp_cols(weff2t, h1st, b2g, xp2t, s * SB * BC, (s + 1) * SB * BC)
            for u in fill_queue:
                emit_fill(u)

            # phase 5: remaining logits
            for v in range(NV):
                todo = [m for m in range(R // 128) if (v, m) not in done_units]
                if not todo:
                    continue
                gt = gbuf.tile([128, 8, 512], dt.bfloat16, tag="gt")
                nc.sync.dma_start(out=gt[:], in_=g12t_d[v])
                for m in todo:
                    logits_unit(v, m, gt)
    return out_d


_CACHED = {}


def _get_compiled():
    if "nc" not in _CACHED:
        from concourse import bacc

        nc = bacc.Bacc(
            "TRN2", target_bir_lowering=False, debug=False, num_devices=NCORES
        )
        build_program(nc)
        nc.compile()
        _CACHED["nc"] = nc
    return _CACHED["nc"]


def kernel(**inputs):
    from concourse.bass_utils import run_bass_kernel_spmd

    per_core, blog = _host_prep(inputs)
    nc = _get_compiled()
    res = run_bass_kernel_spmd(nc, per_core, list(range(NCORES)))
    outs = []
    for c in range(NCORES):
        o = res.results[c]["out"].reshape(T, BC, V)
        outs.append(o.transpose(1, 0, 2))
    out = np.concatenate(outs, axis=0).reshape(B, T, V).astype(np.float32)
    out += blog[None, None, :].astype(np.float32)
    return out
